# revision 1
# baseline (speedup 1.0000x reference)
"""Trainium2 Bass kernel for nn_ExBimamba: bidirectional Mamba block.

Sharding: 8 NeuronCores = 4 samples x 2 directions (fwd/bwd). Each core runs one
full Mamba pass for one (sample, direction) plus its half of the final output
projection; the host sums the two partial projections per sample and adds bo.

Per-core kernel layout: channels on partitions, time on free dim.
- depthwise causal conv as a bf16 tensor-scalar product tree on the DVE, which is
  otherwise idle during the PE-bound input-matmul lead-in
- delta = softplus via ACT Exp+Ln with b_dt as per-partition bias pointer
- dA_n = Exp(A[d,n] * delta) via ACT with per-partition scale pointer
- selective scan via the DVE tensor_tensor_scan instruction, two zero-pad-separated
  (channel-block, n) state segments per instruction
- B/C rows broadcast across partitions with 0-partition-stride DMA (DRAM bounce)
- y = sum_n C*h via identity-stationary accumulating matmuls (PE), with the
  + xh*D skip connection folded in as a diagonal-stationary matmul
"""
import sys
import os

for _p in ('/opt/trn_rl_repo', os.path.join(os.path.dirname(os.path.abspath(__file__)))):
    if _p not in sys.path:
        sys.path.insert(0, _p)

import numpy as np
import ml_dtypes
from contextlib import ExitStack

import concourse.bass as bass
import concourse.bacc as bacc
import concourse.tile as tile
from concourse import mybir
from concourse.bass_utils import run_bass_kernel_spmd

F32 = mybir.dt.float32
BF16 = mybir.dt.bfloat16
AF = mybir.ActivationFunctionType
OP = mybir.AluOpType

B = 4
L = 1024
D_MODEL = 512
D_IN = 1024
N = 16
DT_RANK = 32
K_CONV = 4


def _in_shapes():
    return {
        "xT": ((D_MODEL, L + 4), BF16),
        "w1x": ((D_MODEL, D_IN), BF16),
        "w1z": ((D_MODEL, D_IN), BF16),
        "wx": ((D_IN, 2 * N + DT_RANK), BF16),
        "wdt": ((DT_RANK, D_IN), BF16),
        "wout": ((D_IN, D_MODEL), BF16),
        "wo": ((D_MODEL, D_MODEL), BF16),
        "consts": ((D_IN, N + 3 + K_CONV), F32),
        "ident": ((128, 128), BF16),
        "ddiag": ((D_IN, 128), BF16),
    }


def _kernel_body(tc, out, ins):
    nc = tc.nc
    SEGL = L + 2
    SPI = 2
    QF = SPI * SEGL
    NB = D_IN // 128
    NM = D_MODEL // 128
    TS = 512
    TH = L // TS
    NQ = N // SPI

    with ExitStack() as ctx:
        wpool = ctx.enter_context(tc.tile_pool(name="w", bufs=1))
        pers = ctx.enter_context(tc.tile_pool(name="pers", bufs=1))
        work = ctx.enter_context(tc.tile_pool(name="work", bufs=2))
        spool = ctx.enter_context(tc.tile_pool(name="scan", bufs=2))
        ppool = ctx.enter_context(tc.tile_pool(name="ps", bufs=2, space="PSUM"))
        ypool = ctx.enter_context(tc.tile_pool(name="yps", bufs=1, space="PSUM"))

        def load_rows(name, nchunks, width, dt=BF16, eng=None):
            src = ins[name]
            ts = []
            for c in range(nchunks):
                t = wpool.tile([128, width], dt, tag=f"{name}{c}", name=f"{name}{c}")
                (eng or nc.sync).dma_start(t[:], src[c * 128:(c + 1) * 128, :])
                ts.append(t)
            return ts

        # critical-path loads on the SP queue, in need-order; the rest on ACT's
        xT_sb = load_rows("xT", NM, L + 4)
        cst_sb = load_rows("consts", NB, N + 3 + K_CONV, F32)
        w1x_sb = load_rows("w1x", NM, D_IN)
        wx_sb = load_rows("wx", NB, 2 * N + DT_RANK)
        w1z_sb = load_rows("w1z", NM, D_IN)
        wout_sb = load_rows("wout", NB, D_MODEL)
        wo_sb = load_rows("wo", NM, D_MODEL)
        A_sb = cst_sb
        cb_sb = [t[:, N:N + 1] for t in cst_sb]
        bdt_sb = [t[:, N + 1:N + 2] for t in cst_sb]
        Dp_sb = [t[:, N + 2:N + 3] for t in cst_sb]
        cw_sb = [[t[:, N + 3 + k:N + 4 + k] for k in range(K_CONV)] for t in cst_sb]
        wdt_sb = wpool.tile([DT_RANK, D_IN], BF16)
        nc.sync.dma_start(wdt_sb[:], ins["wdt"][:, :])
        id_sb = wpool.tile([128, 128], BF16)
        nc.sync.dma_start(id_sb[:], ins["ident"][:, :])

        # phase B: xh matmul -> xpre; depthwise conv on DVE (idle in lead-in); silu
        zs_dram = nc.dram_tensor("zs_scratch", [D_IN, L], BF16, kind="Internal").ap()
        xh_sb = [pers.tile([128, L], BF16, tag=f"xh{b}", name=f"xh{b}") for b in range(NB)]
        for b in range(NB):
            xpre = work.tile([128, L + 3], BF16, tag="xpre")
            nc.vector.memset(xpre[:, 0:3], 0.0)
            for th in range(TH):
                ps = ppool.tile([128, TS], F32, tag="pB")
                for cm in range(NM):
                    nc.tensor.matmul(
                        ps[:], w1x_sb[cm][:, b * 128:(b + 1) * 128],
                        xT_sb[cm][:, 3 + th * TS: 3 + th * TS + TS],
                        start=(cm == 0), stop=(cm == NM - 1))
                nc.scalar.copy(xpre[:, 3 + th * TS: 3 + (th + 1) * TS], ps[:])
            tk = []
            for k in range(K_CONV):
                t = work.tile([128, L], BF16, tag=f"ct{k % 2}", bufs=1, name=f"ct{b}_{k}")
                nc.vector.tensor_scalar_mul(t[:], xpre[:, k:k + L], cw_sb[b][k])
                tk.append(t)
                if k % 2 == 1:
                    sm = work.tile([128, L], BF16, tag=f"cs{k // 2}", bufs=1,
                                   name=f"cs{b}_{k}")
                    nc.vector.tensor_add(sm[:], tk[k - 1][:], tk[k][:])
                    tk[k] = sm
            ca = work.tile([128, L], BF16, tag="ct0", bufs=1)
            nc.vector.tensor_add(ca[:], tk[1][:], tk[3][:])
            nc.scalar.activation(xh_sb[b][:], ca[:], AF.Silu, bias=cb_sb[b])

        # phase C: x_dbl = xh @ Wx^T
        dt_sb = pers.tile([DT_RANK, L], BF16)
        bc_sb = pers.tile([2 * N, L], BF16)
        for th in range(TH):
            ps = ppool.tile([2 * N + DT_RANK, TS], F32, tag="pp")
            for b in range(NB):
                nc.tensor.matmul(ps[:], wx_sb[b][:, :], xh_sb[b][:, th * TS:(th + 1) * TS],
                                 start=(b == 0), stop=(b == NB - 1))
            nc.scalar.copy(dt_sb[:, th * TS:(th + 1) * TS], ps[0:DT_RANK, :])
            nc.scalar.copy(bc_sb[:, th * TS:(th + 1) * TS], ps[DT_RANK:2 * N + DT_RANK, :])

        # phase D: broadcast B,C rows across partitions (DRAM bounce, 0-stride read)
        bc_dram = nc.dram_tensor("bc_scratch", [2 * N, L], BF16, kind="Internal").ap()
        nc.sync.dma_start(bc_dram[:, :], bc_sb[:])
        Bbig = pers.tile([128, N * L], BF16)
        Cbig = pers.tile([128, N * L], BF16)
        for n in range(N):
            for big, row, eng in ((Bbig, n, nc.sync), (Cbig, N + n, nc.sync)):
                src = bc_dram[row:row + 1, :]
                src_b = bass.AP(tensor=src.tensor, offset=src.offset,
                                ap=[[0, 128]] + [list(d) for d in src.ap[1:]])
                eng.dma_start(big[:, n * L: (n + 1) * L], src_b)

        # phase B2: z-gate matmuls (emitted after C/D so they don't delay the
        # critical path; PE fills its slack during early phase E)
        for b in range(NB):
            zt = work.tile([128, L], BF16, tag="zt", bufs=1)
            for th in range(TH):
                psz = ppool.tile([128, TS], F32, tag="pB")
                for cm in range(NM):
                    nc.tensor.matmul(
                        psz[:], w1z_sb[cm][:, b * 128:(b + 1) * 128],
                        xT_sb[cm][:, 3 + th * TS: 3 + th * TS + TS],
                        start=(cm == 0), stop=(cm == NM - 1))
                nc.scalar.activation(zt[:, th * TS:(th + 1) * TS], psz[:], AF.Silu)
            nc.sync.dma_start(zs_dram[b * 128:(b + 1) * 128, :], zt[:])

        # phase E: per channel-block: delta, u, dA, scan, y
        y4_sb = [pers.tile([128, L], BF16, tag=f"y4{b}", name=f"y4{b}") for b in range(NB)]
        d0_pp = [spool.tile([128, SPI * SEGL], BF16, tag=f"d0{i}", bufs=1, name=f"d0pp{i}")
                 for i in range(2)]
        d1_pp = [spool.tile([128, SPI * SEGL], BF16, tag=f"d1{i}", bufs=1, name=f"d1pp{i}")
                 for i in range(2)]
        h_pp = [spool.tile([128, SPI * SEGL], BF16, tag=f"h{i}", bufs=1, name=f"hpp{i}")
                for i in range(2)]
        for dd in d0_pp + d1_pp:
            pad = bass.AP(tensor=dd.tensor, offset=dd.offset + L,
                          ap=[list(dd.ap[0]), [SEGL, SPI], [1, SEGL - L]])
            nc.vector.memset(pad, 0.0)
        for b in range(NB):
            zpre = ppool.tile([128, L], F32, tag="zpre", bufs=1)
            for th in range(TH):
                nc.tensor.matmul(zpre[:, th * TS:(th + 1) * TS],
                                 wdt_sb[:, b * 128:(b + 1) * 128],
                                 dt_sb[:, th * TS:(th + 1) * TS],
                                 start=True, stop=True)
            wexp = work.tile([128, L], BF16, tag="wexp", bufs=1)
            nc.scalar.activation(wexp[:], zpre[:], AF.Exp, bias=bdt_sb[b])
            delta = work.tile([128, L], BF16, tag="delta")
            nc.scalar.activation(delta[:], wexp[:], AF.Ln, bias=1.0)
            u = work.tile([128, L], BF16, tag="u", bufs=1)
            nc.vector.tensor_mul(u[:], delta[:], xh_sb[b][:])

            yps = ypool.tile([128, L], F32, tag="yps")
            for q in range(N // SPI):
                d0 = d0_pp[q % 2]
                d1 = d1_pp[q % 2]
                for nn in range(SPI):
                    n = q * SPI + nn
                    nc.scalar.activation(d0[:, nn * SEGL: nn * SEGL + L], delta[:],
                                         AF.Exp, scale=A_sb[b][:, n:n + 1])
                # one fused multiply for both segments: u re-read via 0-stride dim
                d1_out = bass.AP(tensor=d1.tensor, offset=d1.offset,
                                 ap=[list(d1.ap[0]), [SEGL, SPI], [1, L]])
                u_b = bass.AP(tensor=u.tensor, offset=u.offset,
                              ap=[list(u.ap[0]), [0, SPI], [1, L]])
                bslc = Bbig[:, q * SPI * L: (q + 1) * SPI * L]
                b_in = bass.AP(tensor=bslc.tensor, offset=bslc.offset,
                               ap=[list(bslc.ap[0]), [L, SPI], [1, L]])
                nc.vector.tensor_mul(d1_out, u_b, b_in)
                h = h_pp[q % 2]
                nc.vector.tensor_tensor_scan(h[:], d0[:], d1[:], 0.0, OP.mult, OP.add)
                p = spool.tile([128, SPI * L], BF16, tag="p", bufs=1)
                h_in = bass.AP(tensor=h.tensor, offset=h.offset,
                               ap=[list(h.ap[0]), [SEGL, SPI], [1, L]])
                nc.vector.tensor_mul(p[:], h_in, Cbig[:, q * SPI * L:(q + 1) * SPI * L])
                for nn in range(SPI):
                    n = q * SPI + nn
                    for th in range(TH):
                        nc.tensor.matmul(
                            yps[:, th * TS:(th + 1) * TS], id_sb[:],
                            p[:, nn * L + th * TS: nn * L + th * TS + TS],
                            start=(n == 0 and th in (0, 1)), stop=False)
            dd = wpool.tile([128, 128], BF16, tag="ddiag", bufs=2, name=f"dd{b}")
            nc.sync.dma_start(dd[:], ins["ddiag"][b * 128:(b + 1) * 128, :])
            for th in range(TH):
                nc.tensor.matmul(yps[:, th * TS:(th + 1) * TS], dd[:],
                                 xh_sb[b][:, th * TS:(th + 1) * TS],
                                 start=False, stop=True)
            zs = work.tile([128, L], BF16, tag="zs", bufs=1)
            nc.sync.dma_start(zs[:], zs_dram[b * 128:(b + 1) * 128, :])
            ysb = work.tile([128, L], BF16, tag="ysb", bufs=1)
            nc.scalar.copy(ysb[:], yps[:])
            nc.vector.tensor_mul(y4_sb[b][:], ysb[:], zs[:])

        # phase F: mamba out = y4 @ Wout^T
        mo_sb = [pers.tile([128, L], BF16, tag=f"mo{c}", name=f"mo{c}") for c in range(NM)]
        for jm in range(NM):
            for th in range(TH):
                ps = ppool.tile([128, TS], F32, tag="pp")
                for b in range(NB):
                    nc.tensor.matmul(ps[:], wout_sb[b][:, jm * 128:(jm + 1) * 128],
                                     y4_sb[b][:, th * TS:(th + 1) * TS],
                                     start=(b == 0), stop=(b == NB - 1))
                if th == 0:
                    nc.vector.tensor_copy(mo_sb[jm][:, th * TS:(th + 1) * TS], ps[:])
                else:
                    nc.scalar.copy(mo_sb[jm][:, th * TS:(th + 1) * TS], ps[:])

        # phase G: partial final projection = mo @ Wo_half^T
        for jo in range(NM):
            o_sb = work.tile([128, L], F32, tag="osb", bufs=1)
            for th in range(TH):
                ps = ppool.tile([128, TS], F32, tag="pp")
                for cm in range(NM):
                    nc.tensor.matmul(ps[:], wo_sb[cm][:, jo * 128:(jo + 1) * 128],
                                     mo_sb[cm][:, th * TS:(th + 1) * TS],
                                     start=(cm == 0), stop=(cm == NM - 1))
                if th == 0:
                    nc.vector.tensor_copy(o_sb[:, th * TS:(th + 1) * TS], ps[:])
                else:
                    nc.scalar.copy(o_sb[:, th * TS:(th + 1) * TS], ps[:])
            eng = nc.sync if jo % 2 == 0 else nc.scalar
            eng.dma_start(out[jo * 128:(jo + 1) * 128, :], o_sb[:])


_NC_CACHE = None


def _build_nc():
    global _NC_CACHE
    if _NC_CACHE is not None:
        return _NC_CACHE
    nc = bacc.Bacc("TRN2", target_bir_lowering=False, debug=False, num_devices=8)
    ins = {}
    for name, (shape, dt) in _in_shapes().items():
        ins[name] = nc.dram_tensor(name, list(shape), dt, kind="ExternalInput").ap()
    out = nc.dram_tensor("out", [D_MODEL, L], F32, kind="ExternalOutput").ap()
    with tile.TileContext(nc) as tc:
        _kernel_body(tc, out, ins)
    nc.compile()
    _NC_CACHE = nc
    return nc


def _prep_core_inputs(x, p):
    """x: (L, 512) f32 input for this core; p: dict with this direction's params
    plus 'wo_half' (512, 512) = Wo[:, half].T."""
    bf = ml_dtypes.bfloat16
    xTp = np.zeros((D_MODEL, L + 4), np.float32)
    xTp[:, 3:3 + L] = x.T
    W_in = p['W_in']
    conv_w = p['conv_w'][:, 0, :]
    consts = np.concatenate([
        -np.exp(p['A_log']).astype(np.float32),
        p['conv_b'].reshape(-1, 1).astype(np.float32),
        p['b_dt'].reshape(-1, 1).astype(np.float32),
        p['D'].reshape(-1, 1).astype(np.float32),
        conv_w.astype(np.float32)], axis=1)
    return {
        "xT": xTp.astype(bf),
        "w1x": np.ascontiguousarray(W_in[:D_IN, :].T).astype(bf),
        "w1z": np.ascontiguousarray(W_in[D_IN:, :].T).astype(bf),
        "wx": np.ascontiguousarray(p['W_x'].T).astype(bf),
        "wdt": np.ascontiguousarray(p['W_dt'].T).astype(bf),
        "wout": np.ascontiguousarray(p['W_out'].T).astype(bf),
        "wo": np.ascontiguousarray(p['wo_half']).astype(bf),
        "consts": np.ascontiguousarray(consts).astype(np.float32),
        "ident": np.eye(128, dtype=bf),
        "ddiag": np.concatenate([np.diag(p['D'][b * 128:(b + 1) * 128])
                                 for b in range(D_IN // 128)], axis=0).astype(bf),
    }


def _dir_params(inputs, prefix, wo_half):
    names = ['W_in', 'conv_w', 'conv_b', 'W_x', 'W_dt', 'b_dt', 'A_log', 'D', 'W_out']
    p = {n: np.asarray(inputs[prefix + n], np.float32) for n in names}
    p['wo_half'] = wo_half
    return p


def _masked_flip(x, lengths):
    L_ = x.shape[1]
    j = np.arange(L_)[None, :]
    idx = np.where(j < lengths[:, None], lengths[:, None] - 1 - j, j)
    return np.take_along_axis(x, idx[:, :, None], axis=1)


def kernel(**inputs):
    nc = _build_nc()
    hidden = np.asarray(inputs['hidden_input'], np.float32)   # (B, L, 512)
    mask = np.asarray(inputs['mask'], np.int32)
    Wo = np.asarray(inputs['Wo'], np.float32)                 # (512, 1024)
    bo = np.asarray(inputs['bo'], np.float32)

    lengths = mask.sum(axis=1)
    bwd_in = _masked_flip(hidden, lengths)

    pf = _dir_params(inputs, 'f_', np.ascontiguousarray(Wo[:, :D_MODEL].T))
    pb = _dir_params(inputs, 'b_', np.ascontiguousarray(Wo[:, D_MODEL:].T))

    in_maps = []
    for i in range(B):
        in_maps.append(_prep_core_inputs(hidden[i], pf))
    for i in range(B):
        in_maps.append(_prep_core_inputs(bwd_in[i], pb))

    res = run_bass_kernel_spmd(nc, in_maps, core_ids=list(range(8)))

    out = np.empty((B, L, D_MODEL), np.float32)
    for i in range(B):
        fwd = res.results[i]["out"].T                       # (L, 512)
        bwd_f = res.results[B + i]["out"].T                 # (L, 512), flipped time
        bwd = _masked_flip(bwd_f[None], lengths[i:i + 1])[0]
        out[i] = fwd + bwd + bo
    return out

